# revision 38
# baseline (speedup 1.0000x reference)
"""Multi-head attention (RoPE, causal) tensor-parallel over heads on 8 NeuronCores.

Sharding: core c handles heads (2c, 2c+1) for both batch elements.
Each core computes x@Wq/Wk/Wv for its head columns, RoPE, causal flash-style
attention, and a row-sharded o_proj partial [2,2048,1024]; the host sums the
8 partials (the "all-reduce").

Layout strategy (everything transposed so no on-device transposes of x):
  - host passes xT [2, 1024, 2048] (d on partitions)
  - projections produce qT/kT/vT [hd=128(2 heads), tokens] directly
  - S^T tiles [j=128, i=512] = kT.T @ qT per head (K=64, heads row-packed)
  - causal mask added via identity-matmul accumulation of a const mask tile
  - P = exp(S) on ACT (PSUM->SBUF, fp32r out)
  - y^T [65, i] = [V|1].T @ P^T accumulated over j-blocks (rowsum = row 64)
  - normalize with reciprocal + DRAM-bounce partition broadcast + DVE mul
  - o_partial = yT.T @ Wo_rows, one matmul per (token-tile, n-tile)
All matmuls in float32r (tf32-like, ~1.5e-4 rel err, 4x faster than fp32).
"""

import numpy as np

import concourse.bass as bass
import concourse.mybir as mybir
import concourse.tile as tile
from concourse import bacc
from concourse.bass_utils import run_bass_kernel_spmd

F32 = mybir.dt.float32
F32R = mybir.dt.float32r
AF = mybir.ActivationFunctionType

B, L, D, H = 2, 2048, 1024, 16
HD = 64          # head size
NC = 8           # cores
HPC = 2          # heads per core
KT = D // 128    # 8 k-tiles for projections
IW = 512         # i-tile width
NIT = L // IW    # 4 i-tiles
NEG = -1.0e9

_cache = {}


def _build():
    nc = bacc.Bacc("TRN2", target_bir_lowering=False)

    xT = nc.dram_tensor("xT", [B, D, L], F32R, kind="ExternalInput")
    # weights pre-tiled on host: [128, KT, 128] so one contiguous DMA each
    wq = nc.dram_tensor("wq", [128, KT, 128], F32R, kind="ExternalInput")
    wk = nc.dram_tensor("wk", [128, KT, 128], F32R, kind="ExternalInput")
    wv = nc.dram_tensor("wv", [128, KT, 128], F32R, kind="ExternalInput")
    wo = nc.dram_tensor("wo", [128, D], F32R, kind="ExternalInput")
    # rope tables fp32: [128, 2*L] = cos | sinA (sign-folded sin)
    ctab = nc.dram_tensor("ctab", [128, 2 * L], F32, kind="ExternalInput")
    # masks (4 x [128,512]) then identity [128,128]: [128, 4*512+128] f32r
    mtab = nc.dram_tensor("mtab", [128, 4 * IW + 128], F32R, kind="ExternalInput")
    opart = nc.dram_tensor("opart", [B, L, D], F32, kind="ExternalOutput")

    with tile.TileContext(nc) as tc:
        with (
            tc.tile_pool(name="ps", bufs=1, space="PSUM") as psp,
            tc.tile_pool(name="sb", bufs=1) as sbp,
            tc.tile_pool(name="dr", bufs=1, space="DRAM") as drp,
        ):
            # ---- weights / tables; wq + xt first so projections start early
            w_sb = {}
            w_sb["q"] = sbp.tile([128, KT, 128], F32R, tag="wq_t", name="w_q")
            nc.sync.dma_start(w_sb["q"], wq[:])
            xt = [sbp.tile([128, L], F32R, tag="xt", name=f"xt{k}", bufs=8) for k in range(KT)]
            for k in range(KT):
                nc.sync.dma_start(xt[k][:, 0:L // 2], xT[0, k * 128:(k + 1) * 128, 0:L // 2])
                nc.sync.dma_start(xt[k][:, L // 2:], xT[0, k * 128:(k + 1) * 128, L // 2:])
            for name, t in (("k", wk), ("v", wv)):
                w = sbp.tile([128, KT, 128], F32R, tag=f"w{name}_t", name=f"w_{name}")
                nc.sync.dma_start(w, t[:])
                w_sb[name] = w
            wo_sb = sbp.tile([128, D], F32R, tag="wo")
            nc.sync.dma_start(wo_sb, wo[:])
            ct = sbp.tile([128, 2 * L], F32, tag="ctab")
            nc.sync.dma_start(ct, ctab[:])
            cos_t, sin_t = ct[:, 0:L], ct[:, L:2 * L]
            mt = sbp.tile([128, 4 * IW + 128], F32R, tag="mtab")
            nc.sync.dma_start(mt, mtab[:])
            masks = [mt[:, k * IW:(k + 1) * IW] for k in range(4)]
            ident = mt[:, 4 * IW:4 * IW + 128]

            # V_aug layout [128, 16, 130]: per j-tile: VA(64) | 1 | VB(64) | 1
            vaug = [sbp.tile([128, L // 128, 130], F32R, tag="vaug", name=f"vaug{i}", bufs=1)
                     for i in range(2)]
            for v in vaug:
                nc.vector.memset(v[:, :, 64:65].bitcast(F32), 1.0)
                nc.vector.memset(v[:, :, 129:130].bitcast(F32), 1.0)

            qt = sbp.tile([128, L], F32R, tag="qt")
            kt_sb = sbp.tile([128, L], F32R, tag="kt")

            pending_oproj = []

            def flush_oproj():
                for args in pending_oproj:
                    bb, i0_, yt_ = args
                    for tt in range(IW // 128):
                        tok = slice(i0_ + tt * 128, i0_ + tt * 128 + 128)
                        ops = psp.tile([128, 1024], F32, tag="s", bufs=2,
                                       name="ops")
                        for nt in range(2):
                            nc.tensor.matmul(ops[:, nt * 512:(nt + 1) * 512],
                                             yt_[:, tt * 128:tt * 128 + 128],
                                             wo_sb[:, nt * 512:(nt + 1) * 512],
                                             start=True, stop=True)
                        ob = sbp.tile([128, 1024], F32, tag="ob", bufs=3,
                                      name="ob")
                        nc.vector.tensor_copy(ob, ops[:])
                        nc.gpsimd.dma_start(opart[bb, tok, :], ob)
                pending_oproj.clear()

            for b in range(B):
                va = vaug[b % 2]
                if b > 0:
                    for k in range(KT):
                        nc.sync.dma_start(xt[k][:, 0:L // 2],
                                          xT[b, k * 128:(k + 1) * 128, 0:L // 2])
                        nc.sync.dma_start(xt[k][:, L // 2:],
                                          xT[b, k * 128:(k + 1) * 128, L // 2:])

                # ---- q/k projections + rope, interleaved per 512-col chunk
                def proj_chunk(w, it):
                    ps = psp.tile([128, IW], F32, tag="s", bufs=2, name="ps",
                                  padded_shape=[128, 1024])
                    xs = slice(it * IW, (it + 1) * IW)
                    for k in range(KT):
                        nc.tensor.matmul(ps, w[:, k, :], xt[k][:, xs],
                                         start=(k == 0), stop=(k == KT - 1))
                    return ps

                def rope_chunk(ps, dst, it):
                    cs = slice(it * IW, (it + 1) * IW)
                    raw = sbp.tile([128, IW], F32, tag="trraw", bufs=4, name="raw")
                    rot = sbp.tile([128, IW], F32, tag="trrot", bufs=4, name="rot")
                    tsin = sbp.tile([128, IW], F32, tag="trsin", bufs=4, name="tsin")
                    nc.scalar.copy(out=raw, in_=ps[:])
                    r3 = raw.rearrange("(h two) n -> h two n", two=2)
                    o3 = rot.rearrange("(h two) n -> h two n", two=2)
                    nc.sync.dma_start(o3[:, 0, :], r3[:, 1, :])
                    nc.sync.dma_start(o3[:, 1, :], r3[:, 0, :])
                    nc.gpsimd.tensor_mul(tsin, rot, sin_t[:, cs])
                    nc.vector.tensor_mul(rot, raw, cos_t[:, cs])
                    nc.vector.tensor_add(dst[:, cs], rot, tsin)

                vt = sbp.tile([128, L], F32R, tag="vt")
                for it in range(4):
                    psq = proj_chunk(w_sb["q"], it)
                    psk = proj_chunk(w_sb["k"], it)
                    rope_chunk(psq, qt, it)
                    rope_chunk(psk, kt_sb, it)
                for it in range(4):
                    psv = proj_chunk(w_sb["v"], it)
                    nc.any.tensor_copy(vt[:, it * IW:(it + 1) * IW], psv[:])
                for jt in range(L // 128):
                    tp = psp.tile([128, 128], F32R, tag="s", bufs=2,
                                  padded_shape=[128, 1024], name="tp")
                    nc.tensor.transpose(tp, vt[:, jt * 128:(jt + 1) * 128], ident)
                    dst = va[:, jt, :].rearrange("p (a b) -> p a b", b=65)[:, :, 0:64]
                    nc.vector.tensor_copy(dst, tp[:].rearrange("p (a b) -> p a b", a=2))

                # ---- attention per i-tile
                for t in (3, 2, 1, 0):
                    i0 = t * IW
                    nj = 4 * (t + 1)
                    isl = slice(i0, i0 + IW)
                    yps = [psp.tile([65, IW], F32, tag="y", name=f"yps{i}", bufs=4)
                           for i in range(2)]
                    for jb in range(nj):
                        j0 = jb * 128
                        js = slice(j0, j0 + 128)
                        diag = j0 > i0 - 128
                        koff = (j0 - i0) // 128
                        # columns i < j0 are fully masked: restrict to i >= j0
                        off = 128 * koff if diag else 0
                        # both heads in one [128, 1024] psum tile (A | B)
                        sp = psp.tile([128, 1024], F32, tag="s", bufs=2)
                        for hi in range(2):
                            hs = slice(hi * 64, hi * 64 + 64)
                            osl = slice(hi * IW + off, (hi + 1) * IW)
                            nc.tensor.matmul(sp[:, osl], kt_sb[hs, js],
                                             qt[hs, i0 + off:i0 + IW],
                                             start=True, stop=not diag)
                        if diag:
                            for hi in range(2):
                                osl = slice(hi * IW + off, (hi + 1) * IW)
                                nc.tensor.matmul(sp[:, osl], ident,
                                                 masks[koff][:, off:IW],
                                                 start=False, stop=True)
                        pt = sbp.tile([128, 1024], F32R, tag="pt", bufs=3)
                        if off:
                            sp3 = sp[:].rearrange("p (h n) -> p h n", h=2)[:, :, off:IW]
                            pt3 = pt.rearrange("p (h n) -> p h n", h=2)[:, :, off:IW]
                            nc.scalar.activation(pt3, sp3, AF.Exp)
                        else:
                            nc.scalar.activation(pt, sp[:], AF.Exp)
                        st = (jb == 0, jb == nj - 1)
                        nc.tensor.matmul(yps[0][:, off:IW], va[:, jb, 0:65],
                                         pt[:, off:IW],
                                         start=st[0], stop=st[1])
                        nc.tensor.matmul(yps[1][:, off:IW], va[:, jb, 65:130],
                                         pt[:, IW + off:2 * IW],
                                         start=st[0], stop=st[1])

                    # ---- normalize: recip of rowsum, bcast via DRAM, mul
                    yt = sbp.tile([128, IW], F32R, tag="yt", bufs=3)
                    for hi in (1, 0):
                        rs = sbp.tile([65, IW], F32, tag="rsum", bufs=2)
                        nc.vector.reciprocal(rs[64:65, :], yps[hi][64:65, :])
                        rd = drp.tile([1, IW], F32, tag="rdram", bufs=4)
                        nc.sync.dma_start(rd, rs[64:65, :])
                        bc = sbp.tile([64, IW], F32, tag="rbc", bufs=2)
                        nc.sync.dma_start(
                            bc, bass.AP(tensor=rd.tensor, offset=rd.offset,
                                        ap=[[0, 64], [1, IW]]))
                        if hi == 0:
                            nc.vector.tensor_mul(yt[0:64, :], yps[0][0:64, :], bc)
                        else:
                            tmb = sbp.tile([64, IW], F32R, tag="tmb", bufs=2)
                            nc.vector.tensor_mul(tmb, yps[1][0:64, :], bc)
                            nc.sync.dma_start(yt[64:128, :], tmb)

                    # ---- o_proj deferred one i-tile so PE stays on attention
                    pending_oproj.append((b, i0, yt))
                    if len(pending_oproj) > 1:
                        first = pending_oproj.pop(0)
                        rest = pending_oproj[:]
                        pending_oproj.clear()
                        pending_oproj.append(first)
                        flush_oproj()
                        pending_oproj.extend(rest)

            flush_oproj()

    nc.finalize()
    return nc


def _host_inputs(x, Wq, Wk, Wv, Wo):
    xT = np.ascontiguousarray(np.transpose(x, (0, 2, 1)).astype(np.float32))
    # rope tables in transposed layout [dim, pos], tiled for 2 heads
    ts = np.arange(0, HD, 2, dtype=np.float32)
    inv = 10000.0 ** (-ts / HD)                          # [32]
    grid = np.arange(L, dtype=np.float32)[:, None] * inv[None]   # [L, 32]
    sin = np.repeat(np.sin(grid), 2, axis=1).T           # [64, L]
    cos = np.repeat(np.cos(grid), 2, axis=1).T
    sinA = sin.copy()
    sinA[0::2, :] *= -1.0                                # sign fold for pair swap
    ctab = np.concatenate(
        [np.tile(cos, (2, 1)), np.tile(sinA, (2, 1))], axis=1).astype(np.float32)
    ctab = np.ascontiguousarray(ctab)
    # masks: M_k[p, f] = 0 if p <= f - 128k else NEG ; identity
    p = np.arange(128)[:, None]
    f = np.arange(IW)[None, :]
    ms = [np.where(p <= f - 128 * k, 0.0, NEG).astype(np.float32) for k in range(4)]
    mtab = np.concatenate(ms + [np.eye(128, dtype=np.float32)], axis=1)
    mtab = np.ascontiguousarray(mtab)

    scale = 1.0 / float(D) ** 0.5
    maps = []
    for c in range(NC):
        cols = slice(c * 128, (c + 1) * 128)

        def tile_w(w):
            return np.ascontiguousarray(
                w.reshape(KT, 128, 128).transpose(1, 0, 2).astype(np.float32))

        maps.append({
            "xT": xT,
            "wq": tile_w(np.asarray(Wq)[:, cols] * scale),
            "wk": tile_w(np.asarray(Wk)[:, cols]),
            "wv": tile_w(np.asarray(Wv)[:, cols]),
            "wo": np.ascontiguousarray(np.asarray(Wo)[cols, :].astype(np.float32)),
            "ctab": ctab,
            "mtab": mtab,
        })
    return maps


def kernel(x, Wq, Wk, Wv, Wo, num_heads, _trace=False):
    assert int(num_heads) == H
    if "nc" not in _cache:
        _cache["nc"] = _build()
    nc = _cache["nc"]
    maps = _host_inputs(np.asarray(x), Wq, Wk, Wv, Wo)
    res = run_bass_kernel_spmd(nc, maps, core_ids=list(range(NC)), trace=_trace)
    out = res.results[0]["opart"].astype(np.float32).copy()
    for c in range(1, NC):
        out += res.results[c]["opart"]
    if _trace:
        _cache["last_results"] = res
    return out


# revision 39
# speedup vs baseline: 1.0002x; 1.0002x over previous
"""Multi-head attention (RoPE, causal) tensor-parallel over heads on 8 NeuronCores.

Sharding: core c handles heads (2c, 2c+1) for both batch elements.
Each core computes x@Wq/Wk/Wv for its head columns, RoPE, causal flash-style
attention, and a row-sharded o_proj partial [2,2048,1024]; the host sums the
8 partials (the "all-reduce").

Layout strategy (everything transposed so no on-device transposes of x):
  - host passes xT [2, 1024, 2048] (d on partitions)
  - projections produce qT/kT/vT [hd=128(2 heads), tokens] directly
  - S^T tiles [j=128, i=512] = kT.T @ qT per head (K=64, heads row-packed)
  - causal mask added via identity-matmul accumulation of a const mask tile
  - P = exp(S) on ACT (PSUM->SBUF, fp32r out)
  - y^T [65, i] = [V|1].T @ P^T accumulated over j-blocks (rowsum = row 64)
  - normalize with reciprocal + DRAM-bounce partition broadcast + DVE mul
  - o_partial = yT.T @ Wo_rows, one matmul per (token-tile, n-tile)
All matmuls in float32r (tf32-like, ~1.5e-4 rel err, 4x faster than fp32).
"""

import numpy as np

import concourse.bass as bass
import concourse.mybir as mybir
import concourse.tile as tile
from concourse import bacc
from concourse.bass_utils import run_bass_kernel_spmd

F32 = mybir.dt.float32
F32R = mybir.dt.float32r
AF = mybir.ActivationFunctionType

B, L, D, H = 2, 2048, 1024, 16
HD = 64          # head size
NC = 8           # cores
HPC = 2          # heads per core
KT = D // 128    # 8 k-tiles for projections
IW = 512         # i-tile width
NIT = L // IW    # 4 i-tiles
NEG = -1.0e9

_cache = {}


def _build():
    nc = bacc.Bacc("TRN2", target_bir_lowering=False)

    xT = nc.dram_tensor("xT", [B, D, L], F32R, kind="ExternalInput")
    # weights pre-tiled on host: [128, KT, 128] so one contiguous DMA each
    wq = nc.dram_tensor("wq", [128, KT, 128], F32R, kind="ExternalInput")
    wk = nc.dram_tensor("wk", [128, KT, 128], F32R, kind="ExternalInput")
    wv = nc.dram_tensor("wv", [128, KT, 128], F32R, kind="ExternalInput")
    wo = nc.dram_tensor("wo", [128, D], F32R, kind="ExternalInput")
    # rope tables fp32: [128, 2*L] = cos | sinA (sign-folded sin)
    ctab = nc.dram_tensor("ctab", [128, 2 * L], F32, kind="ExternalInput")
    # masks (4 x [128,512]) then identity [128,128]: [128, 4*512+128] f32r
    mtab = nc.dram_tensor("mtab", [128, 4 * IW + 128], F32R, kind="ExternalInput")
    opart = nc.dram_tensor("opart", [B, L, D], F32, kind="ExternalOutput")

    with tile.TileContext(nc) as tc:
        with (
            tc.tile_pool(name="ps", bufs=1, space="PSUM") as psp,
            tc.tile_pool(name="sb", bufs=1) as sbp,
            tc.tile_pool(name="dr", bufs=1, space="DRAM") as drp,
        ):
            # ---- weights / tables; wq + xt first so projections start early
            w_sb = {}
            w_sb["q"] = sbp.tile([128, KT, 128], F32R, tag="wq_t", name="w_q")
            nc.sync.dma_start(w_sb["q"], wq[:])
            xt = [sbp.tile([128, L], F32R, tag="xt", name=f"xt{k}", bufs=8) for k in range(KT)]
            for k in range(KT):
                nc.sync.dma_start(xt[k][:, 0:L // 2], xT[0, k * 128:(k + 1) * 128, 0:L // 2])
                nc.sync.dma_start(xt[k][:, L // 2:], xT[0, k * 128:(k + 1) * 128, L // 2:])
            for name, t in (("k", wk), ("v", wv)):
                w = sbp.tile([128, KT, 128], F32R, tag=f"w{name}_t", name=f"w_{name}")
                nc.sync.dma_start(w, t[:])
                w_sb[name] = w
            wo_sb = sbp.tile([128, D], F32R, tag="wo")
            nc.sync.dma_start(wo_sb, wo[:])
            ct = sbp.tile([128, 2 * L], F32, tag="ctab")
            nc.sync.dma_start(ct, ctab[:])
            cos_t, sin_t = ct[:, 0:L], ct[:, L:2 * L]
            mt = sbp.tile([128, 4 * IW + 128], F32R, tag="mtab")
            nc.sync.dma_start(mt, mtab[:])
            masks = [mt[:, k * IW:(k + 1) * IW] for k in range(4)]
            ident = mt[:, 4 * IW:4 * IW + 128]

            # V_aug layout [128, 16, 130]: per j-tile: VA(64) | 1 | VB(64) | 1
            vaug = [sbp.tile([128, L // 128, 130], F32R, tag="vaug", name=f"vaug{i}", bufs=1)
                     for i in range(2)]
            for v in vaug:
                nc.vector.memset(v[:, :, 64:65].bitcast(F32), 1.0)
                nc.vector.memset(v[:, :, 129:130].bitcast(F32), 1.0)

            qt = sbp.tile([128, L], F32R, tag="qt")
            kt_sb = sbp.tile([128, L], F32R, tag="kt")

            pending_oproj = []

            def flush_oproj():
                for args in pending_oproj:
                    bb, i0_, yt_ = args
                    for tt in range(IW // 128):
                        tok = slice(i0_ + tt * 128, i0_ + tt * 128 + 128)
                        ops = psp.tile([128, 1024], F32, tag="s", bufs=2,
                                       name="ops")
                        for nt in range(2):
                            nc.tensor.matmul(ops[:, nt * 512:(nt + 1) * 512],
                                             yt_[:, tt * 128:tt * 128 + 128],
                                             wo_sb[:, nt * 512:(nt + 1) * 512],
                                             start=True, stop=True)
                        ob = sbp.tile([128, 1024], F32, tag="ob", bufs=3,
                                      name="ob")
                        nc.vector.tensor_copy(ob, ops[:])
                        nc.gpsimd.dma_start(opart[bb, tok, :], ob)
                pending_oproj.clear()

            for b in range(B):
                va = vaug[b % 2]
                if b > 0:
                    for k in range(KT):
                        nc.sync.dma_start(xt[k][:, 0:L // 2],
                                          xT[b, k * 128:(k + 1) * 128, 0:L // 2])
                        nc.sync.dma_start(xt[k][:, L // 2:],
                                          xT[b, k * 128:(k + 1) * 128, L // 2:])

                # ---- q/k projections + rope, interleaved per 512-col chunk
                def proj_chunk(w, it):
                    ps = psp.tile([128, IW], F32, tag="s", bufs=2, name="ps",
                                  padded_shape=[128, 1024])
                    xs = slice(it * IW, (it + 1) * IW)
                    for k in range(KT):
                        nc.tensor.matmul(ps, w[:, k, :], xt[k][:, xs],
                                         start=(k == 0), stop=(k == KT - 1))
                    return ps

                def rope_chunk(ps, dst, it):
                    cs = slice(it * IW, (it + 1) * IW)
                    raw = sbp.tile([128, IW], F32, tag="trraw", bufs=4, name="raw")
                    rot = sbp.tile([128, IW], F32, tag="trrot", bufs=4, name="rot")
                    tsin = sbp.tile([128, IW], F32, tag="trsin", bufs=4, name="tsin")
                    nc.scalar.copy(out=raw, in_=ps[:])
                    r3 = raw.rearrange("(h two) n -> h two n", two=2)
                    o3 = rot.rearrange("(h two) n -> h two n", two=2)
                    nc.sync.dma_start(o3[:, 0, :], r3[:, 1, :])
                    nc.sync.dma_start(o3[:, 1, :], r3[:, 0, :])
                    nc.gpsimd.tensor_mul(tsin, rot, sin_t[:, cs])
                    nc.vector.tensor_mul(rot, raw, cos_t[:, cs])
                    nc.vector.tensor_add(dst[:, cs], rot, tsin)

                vt = sbp.tile([128, L], F32R, tag="vt")
                if b == 0:
                    # k-outer over chunks 0-1 of q and k: consume each xT tile
                    # as it lands during the initial DMA-bound window
                    kps = []
                    for nm, cidx in (("q", 0), ("q", 1), ("k", 0), ("k", 1)):
                        p = psp.tile([128, IW], F32, tag="y", bufs=4,
                                     name=f"kop_{nm}{cidx}")
                        kps.append((nm, cidx, p))
                    for k in range(KT):
                        for nm, cidx, p in kps:
                            xs = slice(cidx * IW, (cidx + 1) * IW)
                            nc.tensor.matmul(p, w_sb[nm][:, k, :], xt[k][:, xs],
                                             start=(k == 0), stop=(k == KT - 1))
                    for nm, cidx, p in kps:
                        rope_chunk(p, qt if nm == "q" else kt_sb, cidx)
                    for it in range(2, 4):
                        psq = proj_chunk(w_sb["q"], it)
                        psk = proj_chunk(w_sb["k"], it)
                        rope_chunk(psq, qt, it)
                        rope_chunk(psk, kt_sb, it)
                else:
                    for it in range(4):
                        psq = proj_chunk(w_sb["q"], it)
                        psk = proj_chunk(w_sb["k"], it)
                        rope_chunk(psq, qt, it)
                        rope_chunk(psk, kt_sb, it)
                for it in range(4):
                    psv = proj_chunk(w_sb["v"], it)
                    nc.any.tensor_copy(vt[:, it * IW:(it + 1) * IW], psv[:])
                for jt in range(L // 128):
                    tp = psp.tile([128, 128], F32R, tag="s", bufs=2,
                                  padded_shape=[128, 1024], name="tp")
                    nc.tensor.transpose(tp, vt[:, jt * 128:(jt + 1) * 128], ident)
                    dst = va[:, jt, :].rearrange("p (a b) -> p a b", b=65)[:, :, 0:64]
                    nc.vector.tensor_copy(dst, tp[:].rearrange("p (a b) -> p a b", a=2))

                # ---- attention per i-tile
                for t in (3, 2, 1, 0):
                    i0 = t * IW
                    nj = 4 * (t + 1)
                    isl = slice(i0, i0 + IW)
                    yps = [psp.tile([65, IW], F32, tag="y", name=f"yps{i}", bufs=4)
                           for i in range(2)]
                    for jb in range(nj):
                        j0 = jb * 128
                        js = slice(j0, j0 + 128)
                        diag = j0 > i0 - 128
                        koff = (j0 - i0) // 128
                        # columns i < j0 are fully masked: restrict to i >= j0
                        off = 128 * koff if diag else 0
                        # both heads in one [128, 1024] psum tile (A | B)
                        sp = psp.tile([128, 1024], F32, tag="s", bufs=2)
                        for hi in range(2):
                            hs = slice(hi * 64, hi * 64 + 64)
                            osl = slice(hi * IW + off, (hi + 1) * IW)
                            nc.tensor.matmul(sp[:, osl], kt_sb[hs, js],
                                             qt[hs, i0 + off:i0 + IW],
                                             start=True, stop=not diag)
                        if diag:
                            for hi in range(2):
                                osl = slice(hi * IW + off, (hi + 1) * IW)
                                nc.tensor.matmul(sp[:, osl], ident,
                                                 masks[koff][:, off:IW],
                                                 start=False, stop=True)
                        pt = sbp.tile([128, 1024], F32R, tag="pt", bufs=3)
                        if off:
                            sp3 = sp[:].rearrange("p (h n) -> p h n", h=2)[:, :, off:IW]
                            pt3 = pt.rearrange("p (h n) -> p h n", h=2)[:, :, off:IW]
                            nc.scalar.activation(pt3, sp3, AF.Exp)
                        else:
                            nc.scalar.activation(pt, sp[:], AF.Exp)
                        st = (jb == 0, jb == nj - 1)
                        nc.tensor.matmul(yps[0][:, off:IW], va[:, jb, 0:65],
                                         pt[:, off:IW],
                                         start=st[0], stop=st[1])
                        nc.tensor.matmul(yps[1][:, off:IW], va[:, jb, 65:130],
                                         pt[:, IW + off:2 * IW],
                                         start=st[0], stop=st[1])

                    # ---- normalize: recip of rowsum, bcast via DRAM, mul
                    yt = sbp.tile([128, IW], F32R, tag="yt", bufs=3)
                    for hi in (1, 0):
                        rs = sbp.tile([65, IW], F32, tag="rsum", bufs=2)
                        nc.vector.reciprocal(rs[64:65, :], yps[hi][64:65, :])
                        rd = drp.tile([1, IW], F32, tag="rdram", bufs=4)
                        nc.sync.dma_start(rd, rs[64:65, :])
                        bc = sbp.tile([64, IW], F32, tag="rbc", bufs=2)
                        nc.sync.dma_start(
                            bc, bass.AP(tensor=rd.tensor, offset=rd.offset,
                                        ap=[[0, 64], [1, IW]]))
                        if hi == 0:
                            nc.vector.tensor_mul(yt[0:64, :], yps[0][0:64, :], bc)
                        else:
                            tmb = sbp.tile([64, IW], F32R, tag="tmb", bufs=2)
                            nc.vector.tensor_mul(tmb, yps[1][0:64, :], bc)
                            nc.sync.dma_start(yt[64:128, :], tmb)

                    # ---- o_proj deferred one i-tile so PE stays on attention
                    pending_oproj.append((b, i0, yt))
                    if len(pending_oproj) > 1:
                        first = pending_oproj.pop(0)
                        rest = pending_oproj[:]
                        pending_oproj.clear()
                        pending_oproj.append(first)
                        flush_oproj()
                        pending_oproj.extend(rest)

            flush_oproj()

    nc.finalize()
    return nc


def _host_inputs(x, Wq, Wk, Wv, Wo):
    xT = np.ascontiguousarray(np.transpose(x, (0, 2, 1)).astype(np.float32))
    # rope tables in transposed layout [dim, pos], tiled for 2 heads
    ts = np.arange(0, HD, 2, dtype=np.float32)
    inv = 10000.0 ** (-ts / HD)                          # [32]
    grid = np.arange(L, dtype=np.float32)[:, None] * inv[None]   # [L, 32]
    sin = np.repeat(np.sin(grid), 2, axis=1).T           # [64, L]
    cos = np.repeat(np.cos(grid), 2, axis=1).T
    sinA = sin.copy()
    sinA[0::2, :] *= -1.0                                # sign fold for pair swap
    ctab = np.concatenate(
        [np.tile(cos, (2, 1)), np.tile(sinA, (2, 1))], axis=1).astype(np.float32)
    ctab = np.ascontiguousarray(ctab)
    # masks: M_k[p, f] = 0 if p <= f - 128k else NEG ; identity
    p = np.arange(128)[:, None]
    f = np.arange(IW)[None, :]
    ms = [np.where(p <= f - 128 * k, 0.0, NEG).astype(np.float32) for k in range(4)]
    mtab = np.concatenate(ms + [np.eye(128, dtype=np.float32)], axis=1)
    mtab = np.ascontiguousarray(mtab)

    scale = 1.0 / float(D) ** 0.5
    maps = []
    for c in range(NC):
        cols = slice(c * 128, (c + 1) * 128)

        def tile_w(w):
            return np.ascontiguousarray(
                w.reshape(KT, 128, 128).transpose(1, 0, 2).astype(np.float32))

        maps.append({
            "xT": xT,
            "wq": tile_w(np.asarray(Wq)[:, cols] * scale),
            "wk": tile_w(np.asarray(Wk)[:, cols]),
            "wv": tile_w(np.asarray(Wv)[:, cols]),
            "wo": np.ascontiguousarray(np.asarray(Wo)[cols, :].astype(np.float32)),
            "ctab": ctab,
            "mtab": mtab,
        })
    return maps


def kernel(x, Wq, Wk, Wv, Wo, num_heads, _trace=False):
    assert int(num_heads) == H
    if "nc" not in _cache:
        _cache["nc"] = _build()
    nc = _cache["nc"]
    maps = _host_inputs(np.asarray(x), Wq, Wk, Wv, Wo)
    res = run_bass_kernel_spmd(nc, maps, core_ids=list(range(NC)), trace=_trace)
    out = res.results[0]["opart"].astype(np.float32).copy()
    for c in range(1, NC):
        out += res.results[c]["opart"]
    if _trace:
        _cache["last_results"] = res
    return out
